# revision 20
# baseline (speedup 1.0000x reference)
"""Trainium2 Bass kernel for nn_DGBasedVonMisesFisherKLD.

Computes okl = mean_j [ logsumexp_i (log_C_kappa + kappa * mu_n[i]@z2[j]) - log A ] - log_C_zero
where mu_n is row-normalized mu [2048, 32], z2 is z reshaped to [65536, 32].

Strategy (per spec sharding hint): shard the j axis (65536) across 8 cores.
mu is replicated. Each core computes, for its 8192 j's:
    S_j = sum_i exp(kappa*m_ij - kappa)   (constant shift is safe: m <= 1)
    partial = sum_j ln(S_j)
via a fused pipeline: TensorE matmul (f32r) -> ScalarE exp+accumulate (in-place
on PSUM) -> final ScalarE ln+accumulate. Host combines 8 tiny partials.

The "- kappa" shift is folded into the matmul as a 33rd contraction row
(zT row 32 = -kappa, muS row 32 = 1), so the activations need no bias APs.
"""

import math
import os
import sys

import numpy as np

if "/opt/trn_rl_repo" not in sys.path:
    sys.path.insert(0, "/opt/trn_rl_repo")

BATCH = 2048
DIM = 32
KDIM = DIM + 1  # contraction rows: 32 data + 1 constant-shift row
N_SAMPLES = 32
N_CORES = 8
J_PER_CORE = BATCH * N_SAMPLES // N_CORES  # 8192
N_JT = J_PER_CORE // 128  # 64 j-tiles of 128
I_CHUNK = 512
N_IC = BATCH // I_CHUNK  # 4 i-chunks of 512

_CACHE = {}


# ---- fallback constants (normally passed in as inputs) ----
def _log_iv(v, x, n_terms=300):
    ks = np.arange(n_terms)
    lg = np.array(
        [math.lgamma(k + 1.0) + math.lgamma(v + k + 1.0) for k in ks]
    )
    logt = (v + 2 * ks) * np.log(x / 2.0) - lg
    m = logt.max()
    return float(m + np.log(np.exp(logt - m).sum()))


def _log_C_d(kappa, d):
    v = d / 2.0 - 1.0
    if kappa == 0.0:
        return float(math.lgamma(d / 2.0) - math.log(2.0) - (d / 2.0) * math.log(math.pi))
    return float(
        v * math.log(kappa) - (d / 2.0) * math.log(2.0 * math.pi) - _log_iv(v, kappa)
    )


def _build_nc(kappa: float, mm_dtype: str):
    """Build the single-core SPMD Bass program (same NEFF on all 8 cores)."""
    import concourse.tile as tile
    from concourse import bacc, mybir

    f32 = mybir.dt.float32
    f32r = mybir.dt.float32r
    mm_dt = f32r if mm_dtype == "f32r" else f32
    AF = mybir.ActivationFunctionType

    nc = bacc.Bacc("TRN2", target_bir_lowering=False, debug=False, num_devices=N_CORES)

    # zT carries [33, J]: rows 0..31 are z2^T, row 32 is the constant -kappa
    zT_d = nc.dram_tensor("zT", [KDIM, J_PER_CORE], mm_dt, kind="ExternalInput").ap()
    muT_d = nc.dram_tensor("muT", [DIM, BATCH], f32, kind="ExternalInput").ap()
    ones_d = nc.dram_tensor("ones_row", [1, BATCH], mm_dt, kind="ExternalInput").ap()
    out_d = nc.dram_tensor("out", [128, 1], f32, kind="ExternalOutput").ap()

    with tile.TileContext(nc) as tc:
        with (
            tc.tile_pool(name="big", bufs=1) as big,
            tc.tile_pool(name="small", bufs=1) as small,
        ):
            # ---- loads ----
            zT = big.tile([KDIM, J_PER_CORE], mm_dt)
            nc.sync.dma_start(zT[:], zT_d[:])
            muT = big.tile([DIM, BATCH], f32)
            nc.sync.dma_start(muT[:], muT_d[:])

            ones_k32 = small.tile([DIM, 1], f32)
            nc.vector.memset(ones_k32[:], 1.0)
            ones_k1 = small.tile([1, DIM], f32)
            nc.vector.memset(ones_k1[:], 1.0)

            # ---- mu normalization (in transposed layout), scaled by kappa ----
            musq = big.tile([DIM, BATCH], f32)
            nc.vector.tensor_tensor(
                out=musq[:], in0=muT[:], in1=muT[:], op=mybir.AluOpType.mult
            )
            muS = big.tile([KDIM, BATCH], mm_dt)  # [kappa*mu_n ; ones], transposed
            nc.sync.dma_start(muS[DIM : DIM + 1, :], ones_d[:])  # the shift row
            acc = small.tile([128, N_JT], f32)  # S_j per (partition, j-tile)

            with tc.tile_pool(name="pp", bufs=1, space="PSUM") as pp:
                # sum of squares per i: ones^T @ musq -> [1, 2048]
                ss = pp.tile([1, BATCH], f32, tag="pre")
                for k in range(N_IC):
                    nc.tensor.matmul(
                        ss[:, k * I_CHUNK : (k + 1) * I_CHUNK],
                        ones_k32[:],
                        musq[:, k * I_CHUNK : (k + 1) * I_CHUNK],
                        start=True,
                        stop=True,
                    )
                # 1 / ||mu_i|| = exp(-0.5*ln(ss)); then * kappa on DVE
                lnss = small.tile([1, BATCH], f32)
                nc.scalar.activation(lnss[:], ss[:], AF.Ln)
                invk = small.tile([1, BATCH], f32)
                nc.scalar.activation(invk[:], lnss[:], AF.Exp, scale=-0.5)
                # broadcast invk across 32 partitions via K=1 matmul
                bc = pp.tile([DIM, BATCH], f32, tag="pre")
                for k in range(N_IC):
                    nc.tensor.matmul(
                        bc[:, k * I_CHUNK : (k + 1) * I_CHUNK],
                        ones_k1[:],
                        invk[:, k * I_CHUNK : (k + 1) * I_CHUNK],
                        start=True,
                        stop=True,
                    )
                # muS = (muT * kappa) * (1/||mu_i||)  in one DVE pass
                nc.vector.scalar_tensor_tensor(
                    out=muS[0:DIM, :],
                    in0=muT[:],
                    scalar=float(kappa),
                    in1=bc[:],
                    op0=mybir.AluOpType.mult,
                    op1=mybir.AluOpType.mult,
                )
                # absorber: fold the zT-DMA completion into the PE vector
                # clock so main-loop matmuls need only (PE, DVE) waits —
                # the ISA allows at most 2 sync waits per instruction.
                warm = pp.tile([1, 16], f32)
                nc.tensor.matmul(
                    warm[:], zT[:, 0:1], zT[:, 0:16], start=True, stop=True
                )

            # ---- main loop: for each j-tile, logits-kappa -> exp -> row-sum ----
            with tc.tile_pool(name="ps", bufs=2, space="PSUM") as ps:
                for t in range(N_JT):
                    P = ps.tile([128, BATCH], f32)
                    for k in range(N_IC):
                        nc.tensor.matmul(
                            P[:, k * I_CHUNK : (k + 1) * I_CHUNK],
                            zT[:, t * 128 : (t + 1) * 128],
                            muS[:, k * I_CHUNK : (k + 1) * I_CHUNK],
                            start=True,
                            stop=True,
                        )
                    # exp(m - kappa), summed over the 2048 i's (free dim)
                    nc.scalar.activation(
                        P[:],
                        P[:],
                        AF.Exp,
                        accum_out=acc[:, t : t + 1],
                    )

            # ---- ln(S_j), summed over j-tiles ----
            lnacc = small.tile([128, N_JT], f32)
            lnsum = small.tile([128, 1], f32)
            nc.scalar.activation(lnacc[:], acc[:], AF.Ln, accum_out=lnsum[:])
            nc.sync.dma_start(out_d[:], lnsum[:])

    nc.finalize()  # Bacc passes: wait-splitting, nop-fusion, act table loads
    return nc


def _get_nc(kappa: float, mm_dtype: str):
    key = (kappa, mm_dtype)
    if key not in _CACHE:
        _CACHE[key] = _build_nc(kappa, mm_dtype)
    return _CACHE[key]


def _install_trace_hook():
    """The image's antenv lacks axon_hooks; shim it so trace=True can ship
    NTFFs back through libaxon_pjrt.so. Safe no-op on failure."""
    try:
        import types

        import antenv

        if "antenv.axon_hooks" not in sys.modules:
            mod = types.ModuleType("antenv.axon_hooks")
            mod._hook = None
            mod.set_axon_ntff_profile_hook = lambda h: setattr(mod, "_hook", h)
            mod.get_axon_ntff_profile_hook = lambda: mod._hook
            sys.modules["antenv.axon_hooks"] = mod
            antenv.axon_hooks = mod
        hooks = sys.modules["antenv.axon_hooks"]
        if hooks.get_axon_ntff_profile_hook() is None:
            from trn_agent_boot.trn_boot import _ntff_profile_via_ctypes

            hooks.set_axon_ntff_profile_hook(
                _ntff_profile_via_ctypes("/opt/axon/libaxon_pjrt.so")
            )
        return True
    except Exception as e:  # pragma: no cover
        print(f"trace hook install failed: {e}")
        return False


def _run(mu, z, kappa, log_C_kappa, log_C_zero, n_samples, trace=False):
    from concourse.bass_utils import run_bass_kernel_spmd

    if trace:
        trace = _install_trace_hook()

    mu = np.ascontiguousarray(np.asarray(mu, dtype=np.float32))
    z = np.ascontiguousarray(np.asarray(z, dtype=np.float32))
    B, d = mu.shape
    n = int(n_samples)
    assert (B, d, n) == (BATCH, DIM, N_SAMPLES), (B, d, n)

    mm_dtype = os.environ.get("BASS_MM_DTYPE", "f32r")
    nc = _get_nc(float(kappa), mm_dtype)

    muT = np.ascontiguousarray(mu.T)
    rows = B // N_CORES
    in_maps = []
    for c in range(N_CORES):
        zc = z[c * rows : (c + 1) * rows].reshape(-1, d)
        zT = np.empty((KDIM, J_PER_CORE), dtype=np.float32)
        zT[:DIM] = zc.T
        zT[DIM] = -float(kappa)
        in_maps.append(
            {
                "zT": zT,
                "muT": muT,
                "ones_row": np.ones((1, BATCH), dtype=np.float32),
            }
        )

    res = run_bass_kernel_spmd(
        nc, in_maps, core_ids=list(range(N_CORES)), trace=trace
    )
    total = sum(float(r["out"].astype(np.float64).sum()) for r in res.results)
    okl = (
        float(log_C_kappa)
        + float(kappa)
        - math.log(B)
        - float(log_C_zero)
        + total / (B * n)
    )
    return np.float32(okl), res


def kernel(
    mu,
    z,
    kappa=100.0,
    log_C_kappa=None,
    log_C_zero=None,
    n_samples=N_SAMPLES,
    **_ignored,
):
    mu = np.asarray(mu)
    if log_C_kappa is None:
        log_C_kappa = _log_C_d(float(kappa), mu.shape[1])
    if log_C_zero is None:
        log_C_zero = _log_C_d(0.0, mu.shape[1])
    okl, _ = _run(mu, z, kappa, log_C_kappa, log_C_zero, n_samples, trace=False)
    return okl
